# revision 32
# baseline (speedup 1.0000x reference)
"""CPPN MLP (12 -> 32 -> 32 -> 32 -> 3, per-node activations) on 8 TRN2 cores.

Data-parallel over the pixel axis; per core 131072 pixels as 4 pixel-groups
on SBUF partitions, feature-major.  All matmuls run in fp16 (measured ~2x
the fp32 rate on silicon; bf16 measured identical speed but 10x worse
accuracy for the identity-node path); PSUM accumulation is fp32.

Per layer the 32 nodes are sorted [tanh-class | gauss | sin] (tanh-class =
tanh/sigmoid/identity, all computed via one Tanh pass with per-partition
scale/bias; sigmoid(z)=0.5 tanh(z/2)+0.5 and identity(z)=tanh(eps z)/eps are
folded into the next layer's weights).  The minor classes (gauss+sin) of TWO
chunks are additionally computed by a small [K,64] matmul into one packed
PSUM tile ([128,1024] = 2 phase-blocks of 64 rows) so their activation
passes run once per two chunks:
The per-node bias is accumulated into the packed tile by a K=1 matmul
(b_eff outer-product with a ones row), so z' = z + b comes out of PSUM:
  sin:   ur = add_range_wrap(z', pi, 2pi)  (one DVE op; wraps one period,
         valid since max|z'| < 3pi; skipped when max|z'| < pi)
         h  = Sin(ur)                   (one ACT pass)
  gauss: y  = Square(0.5 z')            (one ACT pass)
         t  = Tanh(y)                   (one ACT pass)
         den = t + 1                      (one DVE ts)
         r  = 1/den                       (one DVE reciprocal_approx_fast)
         h' = (t-1)*r = -exp(-z'^2/2)     (one DVE STT sub/mult; the
         negation is folded into the next layer's weights)
All passes run full-width [0:128] with junk in the rows of the other class;
the DMA scatter-back copies only the valid rows into each chunk's h tile
(SBUF->SBUF DMA has no partition-alignment constraint, unlike engine APs
which must start at partition 0/32/64/96).
"""

import os
import sys

import numpy as np

_REPO = "/root/.axon_site/_ro/trn_rl_repo"
if _REPO not in sys.path and not os.path.isdir("/opt/trn_rl_repo"):
    sys.path.insert(0, _REPO)

import concourse.bacc as bacc
import concourse.bass as bass  # noqa: F401
import concourse.tile as tile
from concourse import mybir
from concourse.bass_utils import run_bass_kernel_spmd

BF16 = np.float16

# Pin the activation-function table to the single set containing every
# function this kernel uses ({Tanh, Square, Identity, Sin}).  Without this,
# bacc's greedy per-instruction set selection alternates between sets (Sin
# lives only in the trig/silu sets) and emits an ACT_TABLE_LOAD (~1.3us)
# per chunk.
_orig_get_tables = bacc.get_activation_tables


def _pinned_tables(arch):
    t = _orig_get_tables(arch)
    if "silu_and_others" in t:
        return {name: (funcs if name == "silu_and_others" else set())
                for name, funcs in t.items()}
    return t


bacc.get_activation_tables = _pinned_tables

F32 = mybir.dt.float32
BF = mybir.dt.float16

P_TOTAL = 1024 * 1024
N_IN, H, N_OUT = 12, 32, 3
N_CORES = 8
P_CORE = P_TOTAL // N_CORES  # 131072
G = 4                        # pixel groups packed on partitions
PG = P_CORE // G             # 32768 pixels per group per core
CHUNK = 1024                 # pixels per group per chunk
MM_N = 512                   # matmul moving free dim (one PSUM bank)
ID_EPS = np.float32(2.0 ** -7)      # identity-via-tanh input scale
PI = float(np.pi)
TWO_PI = float(2.0 * np.pi)

# class codes: 0 = tanh-class (tanh/sigmoid/identity), 1 = gauss, 2 = sin
def _cls_of_act(a):
    return {3: 2, 4: 1}.get(int(a), 0)


class _Plan:
    """Host-side folded weights + per-layer layouts. All float64 math."""

    def __init__(self, bias_in, W1, b1, act1, W2, b2, act2, W3, b3, act3,
                 Wout, bout, wrap):
        self.wrap = wrap
        layers = [(W1, b1, act1), (W2, b2, act2), (W3, b3, act3)]
        self.nt, self.ng, self.ns = [], [], []
        self.lhsT_main = []     # [K, 128] np.float32 (host; cast to fp16)
        self.lhsT_minor = []    # [K, 64]
        self.bias_rows = []     # [64] minor-bias stationary rows
        self.cols = []          # per-layer dict of [128] operand columns

        # incoming per-node transform: h_true = alpha*stored + beta
        in_alpha = np.ones(N_IN, dtype=np.float64)
        in_beta = np.asarray(bias_in, dtype=np.float64)
        in_dim = N_IN
        # partition of prev-layer node k for group g: L1 input is feature-major
        in_part = [[12 * g + k for k in range(N_IN)] for g in range(G)]

        for li, (W, b, act) in enumerate(layers):
            W = np.asarray(W, dtype=np.float64)
            b = np.asarray(b, dtype=np.float64)
            act = np.asarray(act)
            cls = np.array([_cls_of_act(a) for a in act])
            perm = np.argsort(cls, kind="stable")  # [tanh | gauss | sin]
            nt = int((cls == 0).sum())
            ng = int((cls == 1).sum())
            ns = int((cls == 2).sum())
            assert 4 * (ng + ns) <= 64, (
                f"layer {li}: minor block {4*(ng+ns)} rows > 64; "
                "2-phase packing needs <= 16 minor nodes")
            self.nt.append(nt)
            self.ng.append(ng)
            self.ns.append(ns)

            W_eff = W * in_alpha[:, None]                  # [in_dim, H]
            b_eff = b + in_beta @ W                        # [H]

            # main stationary: node at sorted slot j -> col 4j+g; minor
            # node cols zeroed (their values come from the packed pipeline)
            K = G * in_dim if li == 0 else 128
            lt = np.zeros((K, 128), dtype=np.float64)
            for g in range(G):
                for j in range(nt):
                    node = perm[j]
                    m = 4 * j + g
                    for k in range(in_dim):
                        lt[in_part[g][k], m] = W_eff[k, node]
            self.lhsT_main.append(lt.astype(np.float32))

            # minor stationary: block col layout [gauss 4ng | sin 4ns | pad]
            # (L1: row 48 carries b_eff against the ones-row in the x tile)
            lm = np.zeros((K + 1 if li == 0 else K, 64), dtype=np.float64)
            if li == 0:
                for jg in range(ng):
                    for g in range(G):
                        lm[K, 4 * jg + g] = b_eff[perm[nt + jg]]
                for js in range(ns):
                    for g in range(G):
                        lm[K, 4 * ng + 4 * js + g] = b_eff[perm[nt + ng + js]]
            for g in range(G):
                for jg in range(ng):
                    node = perm[nt + jg]
                    m = 4 * jg + g
                    for k in range(in_dim):
                        lm[in_part[g][k], m] = W_eff[k, node]
                for js in range(ns):
                    node = perm[nt + ng + js]
                    m = 4 * ng + 4 * js + g
                    for k in range(in_dim):
                        lm[in_part[g][k], m] = W_eff[k, node]
            self.lhsT_minor.append(lm.astype(np.float32))

            # operand columns
            m_scale = np.zeros(128, dtype=np.float64)
            m_bias = np.zeros(128, dtype=np.float64)
            sq_scale = np.zeros(128, dtype=np.float64)
            sq_bias = np.zeros(128, dtype=np.float64)
            sin_bias = np.zeros(128, dtype=np.float64)
            bias_row = np.zeros(64, dtype=np.float64)   # minor bias matmul
            out_alpha = np.ones(H, dtype=np.float64)
            out_beta = np.zeros(H, dtype=np.float64)
            for j in range(nt):
                node = perm[j]
                a = int(act[node])
                be = b_eff[node]
                for g in range(G):
                    m = 4 * j + g
                    if a == 1:        # tanh
                        m_scale[m] = 1.0
                        m_bias[m] = be
                    elif a == 2:      # sigmoid -> tanh(z/2)
                        m_scale[m] = 0.5
                        m_bias[m] = 0.5 * be
                    else:             # identity -> tanh(eps*z)
                        m_scale[m] = float(ID_EPS)
                        m_bias[m] = float(ID_EPS) * be
                if a == 1:
                    out_alpha[node], out_beta[node] = 1.0, 0.0
                elif a == 2:
                    out_alpha[node], out_beta[node] = 0.5, 0.5
                else:
                    out_alpha[node], out_beta[node] = 1.0 / float(ID_EPS), 0.0
            # bias location: L1 via the x ones-row; wrapped layers via the
            # K=1 bias matmul; unwrapped others via the ACT bias columns
            bias_in_pm = (li == 0) or wrap[li]
            for ph in range(2):
                for jg in range(ng):
                    node = perm[nt + jg]
                    for g in range(G):
                        m = 64 * ph + 4 * jg + g
                        sq_scale[m] = 0.5
                        if not bias_in_pm:
                            sq_bias[m] = 0.5 * b_eff[node]
                for js in range(ns):
                    node = perm[nt + ng + js]
                    if not bias_in_pm:
                        for g in range(G):
                            sin_bias[64 * ph + 4 * ng + 4 * js + g] = \
                                b_eff[node]
            for jg in range(ng):
                node = perm[nt + jg]
                for g in range(G):
                    bias_row[4 * jg + g] = b_eff[node]
            for js in range(ns):
                node = perm[nt + ng + js]
                for g in range(G):
                    bias_row[4 * ng + 4 * js + g] = b_eff[node]
            for jg in range(ng):
                node = perm[nt + jg]
                out_alpha[node], out_beta[node] = -1.0, 0.0   # stored -exp()
            for js in range(ns):
                node = perm[nt + ng + js]
                out_alpha[node], out_beta[node] = 1.0, 0.0
            gq_bias = np.zeros(128, dtype=np.float64)
            if not bias_in_pm:
                for ph in range(2):
                    for jg in range(ng):
                        node = perm[nt + jg]
                        for g in range(G):
                            gq_bias[64 * ph + 4 * jg + g] = b_eff[node]
            self.cols.append({
                "m_scale": m_scale, "m_bias": m_bias, "sq_scale": sq_scale,
                "sq_bias": sq_bias, "sin_bias": sin_bias, "gq_bias": gq_bias,
            })
            self.bias_rows.append(bias_row.astype(np.float32))

            in_alpha = out_alpha
            in_beta = out_beta
            in_dim = H
            # h layout: node at sorted slot j lives at partition 4j+g
            slot = np.empty(H, dtype=int)
            slot[perm] = np.arange(H)
            in_part = [[4 * slot[k] + g for k in range(H)] for g in range(G)]

        # output layer (quadrant-packed as in the baseline)
        Wo = np.asarray(Wout, dtype=np.float64)
        bo = np.asarray(bout, dtype=np.float64)
        Wo_eff = Wo * in_alpha[:, None]
        bo_eff = bo + in_beta @ Wo
        lt = np.zeros((128, 32), dtype=np.float64)
        for g in range(G):
            for k in range(H):
                for o in range(N_OUT):
                    lt[in_part[g][k], 3 * g + o] = Wo_eff[k, o]
        self.lhsT_out = lt.astype(np.float32)
        out_bias = np.zeros(128, dtype=np.float64)
        for q in range(4):
            for g in range(G):
                for o in range(N_OUT):
                    out_bias[32 * q + 3 * g + o] = bo_eff[o]

        # pack all operand columns into one [128, 16] fp32 block
        colblk = np.zeros((128, 20), dtype=np.float64)
        for li in range(3):
            c = self.cols[li]
            colblk[:, 5 * li + 0] = c["m_scale"]
            colblk[:, 5 * li + 1] = c["m_bias"]
            colblk[:, 5 * li + 2] = c["sin_bias"]
            colblk[:, 5 * li + 3] = c["sq_scale"]
            colblk[:, 5 * li + 4] = c["sq_bias"]
            colblk[:, 16 + li] = c["gq_bias"]
        colblk[:, 15] = out_bias
        self.colblk = colblk.astype(np.float32)

    def key(self):
        return tuple(zip(self.nt, self.ng, self.ns))


def _build_program(nt, ng, ns, wrap, p_core=P_CORE, chunk=CHUNK):
    """Build the bass module. Program structure depends only on the per-layer
    (nt, ng, ns) counts, not on weight values."""
    pg = p_core // G
    nchunk = pg // chunk
    nblock = nchunk // 2
    nhalf = chunk // MM_N
    assert chunk % MM_N == 0 and pg % chunk == 0 and nchunk % 2 == 0

    nc = bacc.Bacc("TRN2", target_bir_lowering=False, debug=False,
                   num_devices=N_CORES)
    xT = nc.dram_tensor("xT", [G * N_IN + 1, pg], BF,
                        kind="ExternalInput").ap()
    wb = nc.dram_tensor("wb", [128, 800], BF, kind="ExternalInput").ap()
    ones = nc.dram_tensor("ones", [1, chunk], BF, kind="ExternalInput").ap()
    cc = nc.dram_tensor("cc", [128, 20], F32, kind="ExternalInput").ap()
    yT = nc.dram_tensor("yT", [12, pg], F32, kind="ExternalOutput").ap()

    A = mybir.AluOpType
    AF = mybir.ActivationFunctionType

    with tile.TileContext(nc) as tc:
        cpool = tc.alloc_tile_pool(name="consts", bufs=1)
        wst_t = cpool.tile([128, 800], BF, tag="wst")
        cc_t = cpool.tile([128, 20], F32, tag="cc")
        ones_t = cpool.tile([1, chunk], BF, tag="ones")
        nc.sync.dma_start(out=wst_t[:], in_=wb[:])
        nc.sync.dma_start(out=cc_t[:], in_=cc[:])
        nc.sync.dma_start(out=ones_t[:], in_=ones[:])
        wmain = [wst_t[:, 128 * li:128 * (li + 1)] for li in range(3)]
        wminor = [wst_t[:, 384 + 64 * li:384 + 64 * (li + 1)]
                  for li in range(3)]
        wout = wst_t[:, 576:608]
        wbias = [wst_t[0:1, 608 + 64 * li:608 + 64 * (li + 1)]
                 for li in range(3)]

        xpool = tc.alloc_tile_pool(name="xin", bufs=8)
        hpool = [tc.alloc_tile_pool(name=f"h{li}", bufs=8) for li in range(3)]
        upool = tc.alloc_tile_pool(name="ur", bufs=2)
        ypool = tc.alloc_tile_pool(name="ysq", bufs=2)
        tpool = tc.alloc_tile_pool(name="tg", bufs=2)
        mpool = tc.alloc_tile_pool(name="hmin", bufs=6)
        opool = tc.alloc_tile_pool(name="osb", bufs=2)
        pmain = tc.alloc_tile_pool(name="ps_main", bufs=3, space="PSUM")
        pminor = tc.alloc_tile_pool(name="ps_minor", bufs=2, space="PSUM")
        pout = tc.alloc_tile_pool(name="ps_out", bufs=1, space="PSUM")

        h_live = {}      # (c, li) -> tile (li 0 == x input)
        pm_live = {}     # (b, li) -> packed minor psum tile
        pso_live = {}    # chunk-pair -> out psum tile

        def emit_load(c):
            x_t = xpool.tile([G * N_IN + 1, chunk], BF, tag="x")
            nc.gpsimd.dma_start(out=x_t[:],
                                in_=xT[:, c * chunk:(c + 1) * chunk])
            h_live[(c, 0)] = x_t

        def emit_layer_pair(bk, li):
            """Both chunks' main+minor matmuls for one layer, batched so the
            PE runs 4 same-stationary matmuls back-to-back per LDWEIGHTS."""
            kdim = G * N_IN if li == 0 else 128
            kmin = kdim + 1 if li == 0 else kdim  # L1 minor uses the ones-row
            bias_mm = wrap[li] and li > 0
            cb = 5 * li
            pm = pminor.tile([128, chunk], F32, tag="pmin", name=f"pm{li}")
            pm_live[(bk, li)] = pm
            chunks = (2 * bk, 2 * bk + 1)
            hprev = [h_live[(c, li)] for c in chunks]
            hnew = [hpool[li].tile([128, chunk], BF, tag="h", name=f"h{p}")
                    for p in range(2)]
            pss = [[pmain.tile([128, MM_N], F32, tag="pre", name=f"ps{p}{hh}")
                    for hh in range(nhalf)] for p in range(2)]
            for p in range(2):
                for hh in range(nhalf):
                    sl = slice(hh * MM_N, (hh + 1) * MM_N)
                    nc.tensor.matmul(
                        pss[p][hh][:], wmain[li][0:kdim, :],
                        hprev[p][0:kdim, sl], start=True, stop=True,
                    )
            for p in range(2):
                for hh in range(nhalf):
                    sl = slice(hh * MM_N, (hh + 1) * MM_N)
                    # tanh-class pass (junk on minor rows; overwritten by
                    # the scatter-back DMAs)
                    nc.scalar.activation(
                        hnew[p][:, sl], pss[p][hh][:], AF.Tanh,
                        bias=cc_t[:, cb + 1:cb + 2],
                        scale=cc_t[:, cb + 0:cb + 1],
                    )
            for p in range(2):
                for hh in range(nhalf):
                    sl = slice(hh * MM_N, (hh + 1) * MM_N)
                    nc.tensor.matmul(
                        pm[64 * p:64 * (p + 1), sl],
                        wminor[li][0:kmin, :], hprev[p][0:kmin, sl],
                        start=True, stop=not bias_mm,
                        tile_position=(0, 64 * p),
                    )
                if bias_mm:
                    # close this phase's accumulation group before the next
                    # phase opens one in the same PSUM banks
                    for hh in range(nhalf):
                        sl = slice(hh * MM_N, (hh + 1) * MM_N)
                        nc.tensor.matmul(
                            pm[64 * p:64 * (p + 1), sl],
                            wbias[li], ones_t[0:1, sl],
                            start=False, stop=True,
                            tile_position=(0, 64 * p),
                        )
            for p in range(2):
                h_live[(chunks[p], li + 1)] = hnew[p]

        def emit_minor(bk, li):
            """Packed minor pipeline for block bk (chunks 2bk, 2bk+1)."""
            pm = pm_live.pop((bk, li))
            cb = 5 * li
            lng, lns = ng[li], ns[li]
            hs = hg = None
            if lns:
                # sin: one-period range wrap (max|z'| < 3pi), skipped when
                # max|z'| < pi already
                if wrap[li]:
                    ur = upool.tile([128, chunk], F32, tag="ur")
                    nc.vector.add_range_wrap(ur[:], pm[:], 0.0, PI, TWO_PI)
                    sin_src = ur
                else:
                    sin_src = pm
                hs = mpool.tile([128, chunk], BF, tag="hs")
                if wrap[li] or li == 0:
                    nc.scalar.activation(hs[:], sin_src[:], AF.Sin)
                else:
                    nc.scalar.activation(hs[:], sin_src[:], AF.Sin,
                                         bias=cc_t[:, cb + 2:cb + 3])
            if lng:
                y_t = ypool.tile([128, chunk], F32, tag="ysq")
                nc.scalar.activation(
                    y_t[:], pm[:], AF.Square,
                    bias=cc_t[:, cb + 4:cb + 5],
                    scale=cc_t[:, cb + 3:cb + 4],
                )
                t_t = tpool.tile([128, chunk], BF, tag="tg")
                nc.scalar.activation(t_t[:], y_t[:], AF.Tanh)
                den = ypool.tile([128, chunk], F32, tag="den")
                nc.vector.tensor_scalar(
                    den[:], t_t[:], 1.0, None, A.add)
                rin = ypool.tile([128, chunk], F32, tag="rin")
                nc.vector.reciprocal_approx_fast(rin[:], den[:])
                hg = mpool.tile([128, chunk], BF, tag="hg")
                nc.vector.scalar_tensor_tensor(
                    hg[:], t_t[:], 1.0, rin[:], A.subtract, A.mult)
            # scatter valid rows back into each chunk's h tile
            for ph in range(2):
                c = 2 * bk + ph
                h = h_live[(c, li + 1)]
                base = 64 * ph
                if lng:
                    nc.gpsimd.dma_start(
                        out=h[4 * nt[li]:4 * (nt[li] + lng), :],
                        in_=hg[base:base + 4 * lng, :])
                if lns:
                    nc.sync.dma_start(
                        out=h[4 * (nt[li] + lng):4 * (nt[li] + lng + lns), :],
                        in_=hs[base + 4 * lng:base + 4 * (lng + lns), :])

        def emit_out(c):
            h_prev = h_live.pop((c, 3))
            h_live.pop((c, 0))
            q0 = 2 * (c % 2)
            if q0 == 0:
                pso_live[c // 2] = pout.tile([128, MM_N], F32, tag="preo",
                                             name="pso")
            pso = pso_live[c // 2]
            for hh in range(nhalf):
                q = q0 + hh
                nc.tensor.matmul(
                    pso[32 * q:32 * q + 32, :],
                    wout, h_prev[:, hh * MM_N:(hh + 1) * MM_N],
                    start=True, stop=True,
                    tile_position=(0, 32 * q),
                )
            if q0 == 2:
                pso_live.pop(c // 2)
                osb = opool.tile([128, MM_N], F32, tag="osb")
                nc.scalar.activation(
                    osb[:], pso[:], AF.Tanh, bias=cc_t[:, 15:16])
                base = (c - 1) * chunk
                for q in range(4):
                    nc.sync.dma_start(
                        out=yT[:, base + q * MM_N: base + (q + 1) * MM_N],
                        in_=osb[32 * q:32 * q + 12, :])

        # Block-granular software pipeline: at step t emit
        #   load(2t, 2t+1), L1-mm(2t, 2t+1), L1-minor(t),
        #   L2-mm/minor(t-1), L3-mm/minor(t-2), out(t-3).
        for t in range(nblock + 3):
            if t < nblock:
                emit_load(2 * t)
                emit_load(2 * t + 1)
                emit_layer_pair(t, 0)
                emit_minor(t, 0)
            if 1 <= t and t - 1 < nblock:
                emit_layer_pair(t - 1, 1)
                emit_minor(t - 1, 1)
            if 2 <= t and t - 2 < nblock:
                emit_layer_pair(t - 2, 2)
                emit_minor(t - 2, 2)
            if 3 <= t and t - 3 < nblock:
                emit_out(2 * (t - 3))
                emit_out(2 * (t - 3) + 1)

        for p in (pout, pminor, pmain, opool, mpool, tpool, ypool, upool,
                  hpool[2], hpool[1], hpool[0], xpool, cpool):
            p.release()

    nc.compile()
    return nc


_PROGRAM_CACHE = {}


def _get_program(key, nt, ng, ns, wrap, p_core=P_CORE, chunk=CHUNK):
    k = (key, tuple(wrap), p_core, chunk)
    if k not in _PROGRAM_CACHE:
        _PROGRAM_CACHE[k] = _build_program(nt, ng, ns, wrap, p_core, chunk)
    return _PROGRAM_CACHE[k]


def make_in_maps(inputs, plan, p_core=P_CORE, n_cores=N_CORES):
    """Shard + transpose the pixel data; replicate constants."""
    x = np.asarray(inputs["inputs"], dtype=np.float32)
    pg = p_core // G
    wbp = np.zeros((128, 800), dtype=np.float32)
    for li in range(3):
        lt = plan.lhsT_main[li]
        wbp[0:lt.shape[0], 128 * li:128 * li + 128] = lt
        lm = plan.lhsT_minor[li]
        wbp[0:lm.shape[0], 384 + 64 * li:384 + 64 * li + 64] = lm
        wbp[0, 608 + 64 * li:608 + 64 * li + 64] = plan.bias_rows[li]
    wbp[:, 576:608] = plan.lhsT_out
    wbp16 = wbp.astype(BF16)
    ones = np.ones((1, CHUNK), dtype=BF16)
    in_maps = []
    for core in range(n_cores):
        xc = x[core * p_core:(core + 1) * p_core]          # [p_core, 12]
        xg = xc.reshape(G, pg, N_IN)                        # [G, pg, 12]
        xT = np.ones((G * N_IN + 1, pg), dtype=BF16)
        xT[0:G * N_IN] = xg.transpose(0, 2, 1).reshape(G * N_IN, pg)
        in_maps.append({"xT": xT, "wb": wbp16, "cc": plan.colblk,
                        "ones": ones})
    return in_maps


def assemble_output(results, p_core=P_CORE, n_cores=N_CORES):
    pg = p_core // G
    out = np.empty((p_core * n_cores, N_OUT), dtype=np.float32)
    for core in range(n_cores):
        yT = np.asarray(results[core]["yT"], dtype=np.float32)  # [12, pg]
        yc = yT.reshape(G, N_OUT, pg).transpose(0, 2, 1)        # [G, pg, 3]
        out[core * p_core:(core + 1) * p_core] = yc.reshape(p_core, N_OUT)
    return out


def _sin_ranges(inputs):
    """Per-layer max |z+b| over sin nodes (fp32 forward on the real data)."""
    x = np.asarray(inputs["inputs"], np.float32)
    h = x + np.asarray(inputs["bias_in"], np.float32)
    mx = []
    for li in (1, 2, 3):
        W = np.asarray(inputs[f"W{li}"], np.float32)
        b = np.asarray(inputs[f"b{li}"], np.float32)
        act = np.asarray(inputs[f"act{li}"])
        pre = h @ W + b
        sel = act == 3
        mx.append(float(np.abs(pre[:, sel]).max()) if sel.any() else 0.0)
        out = np.empty_like(pre)
        for k in range(5):
            m = act == k
            if not m.any():
                continue
            f = [lambda v: v, np.tanh,
                 lambda v: 1.0 / (1.0 + np.exp(-v)), np.sin,
                 lambda v: np.exp(-0.5 * v * v)][k]
            out[:, m] = f(pre[:, m])
        h = out
    return mx


def make_plan(inputs):
    mx = _sin_ranges(inputs)
    # 2% slack for fp16 weight/activation perturbation of the pre-acts
    assert all(m < 3 * PI * 0.98 for m in mx), f"sin pre-acts {mx} >= 3pi"
    wrap = [m >= PI * 0.98 for m in mx]
    return _Plan(
        inputs["bias_in"], inputs["W1"], inputs["b1"], inputs["act1"],
        inputs["W2"], inputs["b2"], inputs["act2"],
        inputs["W3"], inputs["b3"], inputs["act3"],
        inputs["Wout"], inputs["bout"], wrap)


def run(inputs, trace=False, **spmd_kwargs):
    plan = make_plan(inputs)
    nc = _get_program(plan.key(), plan.nt, plan.ng, plan.ns, plan.wrap)
    in_maps = make_in_maps(inputs, plan)
    res = run_bass_kernel_spmd(nc, in_maps, list(range(N_CORES)),
                               trace=trace, **spmd_kwargs)
    return assemble_output(res.results), res


def kernel(**inputs) -> np.ndarray:
    out, _ = run(inputs, trace=False)
    return out
